# revision 7
# baseline (speedup 1.0000x reference)
"""Contrastive loss (GRACE-style) on 8 Trainium2 NeuronCores.

loss = sum_i 0.5*(l1_i + l2_i)
  l1 = -log(diag(exp(h1@h2.T/t)) / (rowsum(exp(h1@h1.T/t)) + rowsum(exp(h1@h2.T/t)) - diag(exp(h1@h1.T/t))))
  l2 = same with h1<->h2;  h = z / ||z||_row,  t = 0.2

Sharding: columns (j) of the similarity matrices are sharded across 8 cores
(each core owns a 1024-column chunk of both h1 and h2). Each core computes,
for ALL 8192 rows i, the partial sums over its j-chunk of
exp(s_i * (z_i . h_j) / t), where h_j is the normalized chunk column and the
row normalization s_i/t is applied inside the ACT exp via a per-partition
scale vector. refl and between column blocks are concatenated into one
[512, 2048] rhs so one ACT instruction accumulates refl+between partial
row-sums together. Host sums the 8 partials in fp64 and applies logs.
"""

import numpy as np
import ml_dtypes

N = 8192
D = 512
NCORES = 8
CH = N // NCORES  # 1024 columns per core
P = 128
KD = D // P  # 4 contraction tiles
NIB = N // P  # 64 i-blocks
NCT = CH // P  # 8 chunk row-tiles
NZC = 8  # zt column chunks (of 1024) per kd tile
TAU_INV = 5.0

_CACHE = {}


def _build(repeat=1, loop=None):
    import concourse.tile as tile
    from concourse import bacc, mybir
    from concourse.masks import make_identity

    f32 = mybir.dt.float32
    bf16 = mybir.dt.bfloat16
    AF = mybir.ActivationFunctionType
    ALU = mybir.AluOpType

    nc = bacc.Bacc("TRN2", target_bir_lowering=False, debug=False,
                   num_devices=NCORES)

    z1t = nc.dram_tensor("z1t", [D, N], bf16, kind="ExternalInput")
    z2t = nc.dram_tensor("z2t", [D, N], bf16, kind="ExternalInput")
    z1r = nc.dram_tensor("z1r", [N, D], bf16, kind="ExternalInput")
    z2r = nc.dram_tensor("z2r", [N, D], bf16, kind="ExternalInput")
    z1c = nc.dram_tensor("z1c", [CH, D], bf16, kind="ExternalInput")
    z2c = nc.dram_tensor("z2c", [CH, D], bf16, kind="ExternalInput")
    partials = nc.dram_tensor("partials", [2, N], f32, kind="ExternalOutput")
    diag = nc.dram_tensor("diag", [CH], f32, kind="ExternalOutput")
    ecol = nc.dram_tensor("ecol", [CH], f32, kind="ExternalOutput")

    z1t_v = z1t.rearrange("(k p) n -> p k n", p=P)
    z2t_v = z2t.rearrange("(k p) n -> p k n", p=P)

    with tile.TileContext(nc) as tc:
        with (
            tc.tile_pool(name="singles", bufs=1) as singles,
            tc.tile_pool(name="zr", bufs=4) as zrp,
            tc.tile_pool(name="scr", bufs=3) as scrp,
            tc.tile_pool(name="h", bufs=3) as hp,
            tc.tile_pool(name="es", bufs=2) as esp,
            tc.tile_pool(name="ps", bufs=3, space="PSUM") as psp,
            tc.tile_pool(name="pscol", bufs=1, space="PSUM") as pscolp,
        ):
            ident = singles.tile([P, P], bf16, tag="ident")
            make_identity(nc, ident)

            # ---- persistent buffers ----
            # stationary operands, 64 chunk tiles [128, 1024]
            zt_tiles = {}
            for nm in ("zt1", "zt2"):
                zt_tiles[nm] = [
                    [singles.tile([P, CH], bf16, tag=f"{nm}_{kd}_{c}",
                                  name=f"{nm}_{kd}_{c}")
                     for c in range(NZC)]
                    for kd in range(KD)
                ]
            rhs = singles.tile([P, KD, 2 * CH], bf16, tag="rhs")
            sq1 = singles.tile([P, NIB], f32, tag="sq1")
            sq2 = singles.tile([P, NIB], f32, tag="sq2")
            s1tau = singles.tile([P, NIB], f32, tag="s1tau")
            s2tau = singles.tile([P, NIB], f32, tag="s2tau")
            stmp = singles.tile([P, NIB], f32, tag="stmp")
            stmp2 = singles.tile([P, NIB], f32, tag="stmp2")
            acc1 = singles.tile([P, NIB], f32, tag="acc1")
            acc1b = singles.tile([P, NIB], f32, tag="acc1b")
            acc2 = singles.tile([P, NIB], f32, tag="acc2")
            ones = singles.tile([P, 1], bf16, tag="ones")
            nc.vector.memset(ones, 1.0)
            ecol_s = singles.tile([1, CH], f32, tag="ecol_s")
            cs0 = pscolp.tile([1, 512], f32, tag="cs0")
            cs1 = pscolp.tile([1, 512], f32, tag="cs1")
            sqc = singles.tile([P, 2 * NCT], f32, tag="sqc")
            sctmp = singles.tile([P, 2 * NCT], f32, tag="sctmp")
            sc = singles.tile([P, 2 * NCT], f32, tag="sc")
            dotc = singles.tile([P, NCT], f32, tag="dotc")
            v5a = singles.tile([P, NCT], f32, tag="v5a")
            v5 = singles.tile([P, NCT], f32, tag="v5")
            c1_tiles = [singles.tile([P, D], bf16, tag=f"c1_{t}",
                                     name=f"c1_{t}") for t in range(NCT)]
            c2_tiles = [singles.tile([P, D], bf16, tag=f"c2_{t}",
                                     name=f"c2_{t}") for t in range(NCT)]

            # ---- DMA: chunk rows first (gate everything local) ----
            for t in range(NCT):
                nc.sync.dma_start(out=c1_tiles[t],
                                  in_=z1c[t * P:(t + 1) * P, :])
                nc.sync.dma_start(out=c2_tiles[t],
                                  in_=z2c[t * P:(t + 1) * P, :])

            # zt1 first chunks so pass-1 matmuls can start early
            for c in range(2):
                for kd in range(KD):
                    nc.sync.dma_start(
                        out=zt_tiles["zt1"][kd][c],
                        in_=z1t_v[:, kd, c * CH:(c + 1) * CH])

            # ---- chunk norms + cross dots (DVE) ----
            for t in range(NCT):
                s = scrp.tile([P, D], f32, tag="scr")
                nc.vector.tensor_mul(s, c1_tiles[t], c1_tiles[t])
                nc.vector.tensor_reduce(sqc[:, t:t + 1], s,
                                        axis=mybir.AxisListType.X, op=ALU.add)
                s = scrp.tile([P, D], f32, tag="scr")
                nc.vector.tensor_mul(s, c2_tiles[t], c2_tiles[t])
                nc.vector.tensor_reduce(sqc[:, NCT + t:NCT + t + 1], s,
                                        axis=mybir.AxisListType.X, op=ALU.add)
                s = scrp.tile([P, D], f32, tag="scr")
                nc.vector.tensor_mul(s, c1_tiles[t], c2_tiles[t])
                nc.vector.tensor_reduce(dotc[:, t:t + 1], s,
                                        axis=mybir.AxisListType.X, op=ALU.add)

            # sc = 1/sqrt(sqc)
            nc.scalar.activation(out=sctmp, in_=sqc, func=AF.Sqrt)
            nc.vector.reciprocal(sc, sctmp)

            # v5 = dotc * sc1 * sc2 * (1/tau)  ( = ln between_ii for own rows)
            nc.vector.tensor_mul(v5a, dotc, sc[:, 0:NCT])
            nc.vector.tensor_mul(v5a, v5a, sc[:, NCT:2 * NCT])
            nc.vector.tensor_scalar_mul(v5, v5a, TAU_INV)
            nc.sync.dma_start(out=diag.rearrange("(t p) -> p t", p=P), in_=v5)

            # ---- normalize chunk + transpose into rhs ----
            for t in range(NCT):
                for half, (ct, scol) in enumerate(
                        [(c1_tiles[t], t), (c2_tiles[t], NCT + t)]):
                    h = hp.tile([P, D], bf16, tag="h")
                    nc.scalar.mul(h, ct, sc[:, scol:scol + 1])
                    for kd in range(KD):
                        pst = psp.tile([P, P], bf16, tag="ps")
                        nc.tensor.transpose(pst, h[:, kd * P:(kd + 1) * P],
                                            ident)
                        nc.vector.tensor_copy(
                            rhs[:, kd, half * CH + t * P: half * CH + (t + 1) * P],
                            pst)

            # ---- full row norms: z1 (needed before first exp) ----
            for t in range(NIB):
                zr = zrp.tile([P, D], bf16, tag="zr")
                nc.sync.dma_start(out=zr, in_=z1r[t * P:(t + 1) * P, :])
                s = scrp.tile([P, D], f32, tag="scr")
                nc.vector.tensor_mul(s, zr, zr)
                nc.vector.tensor_reduce(sq1[:, t:t + 1], s,
                                        axis=mybir.AxisListType.X, op=ALU.add)
            nc.scalar.activation(out=stmp, in_=sq1, func=AF.Sqrt)
            nc.vector.reciprocal(stmp, stmp)
            nc.vector.tensor_scalar_mul(s1tau, stmp, TAU_INV)

            # rest of zt1
            for c in range(2, NZC):
                for kd in range(KD):
                    nc.sync.dma_start(
                        out=zt_tiles["zt1"][kd][c],
                        in_=z1t_v[:, kd, c * CH:(c + 1) * CH])

            # z2 row norms (needed before pass 2 exps)
            for t in range(NIB):
                zr = zrp.tile([P, D], bf16, tag="zr")
                nc.sync.dma_start(out=zr, in_=z2r[t * P:(t + 1) * P, :])
                s = scrp.tile([P, D], f32, tag="scr")
                nc.vector.tensor_mul(s, zr, zr)
                nc.vector.tensor_reduce(sq2[:, t:t + 1], s,
                                        axis=mybir.AxisListType.X, op=ALU.add)
            nc.scalar.activation(out=stmp2, in_=sq2, func=AF.Sqrt)
            nc.vector.reciprocal(stmp2, stmp2)
            nc.vector.tensor_scalar_mul(s2tau, stmp2, TAU_INV)

            # zt2
            for c in range(NZC):
                for kd in range(KD):
                    nc.sync.dma_start(
                        out=zt_tiles["zt2"][kd][c],
                        in_=z2t_v[:, kd, c * CH:(c + 1) * CH])

            # ---- main ----
            # pass 1: z1t x [h1cT | h2cT]; refl1+between1 partial rowsums,
            #   plus colsums of exp'd between tiles (= full between2 rowsums
            #   for own chunk rows) via ones-matmuls accumulated in PSUM.
            # pass 2: z2t x h2cT only (refl2 partial rowsums).
            def _main_body():
                esb_pend = []

                def _colsum(item, ib0=None, ib63=None):
                    ibx, esb = item
                    for half, cs in ((0, cs0), (1, cs1)):
                        nc.tensor.matmul(
                            cs[0:1, :],
                            lhsT=ones,
                            rhs=esb[:, half * 512:(half + 1) * 512],
                            start=(ibx == 0),
                            stop=(ibx == NIB - 1),
                            skip_group_check=True,
                        )

                for ib in range(NIB):
                    c, lb = divmod(ib, NZC)
                    psa = psp.tile([P, 2 * 512], f32, tag="ps", name="psa")
                    psb = psp.tile([P, 2 * 512], f32, tag="ps", name="psb")
                    for kd in range(KD):
                        lhsT = zt_tiles["zt1"][kd][c][:, lb * P:(lb + 1) * P]
                        for jt in range(2):
                            nc.tensor.matmul(
                                psa[:, jt * 512:(jt + 1) * 512],
                                lhsT=lhsT,
                                rhs=rhs[:, kd, jt * 512:(jt + 1) * 512],
                                start=(kd == 0), stop=(kd == KD - 1))
                        for jt in range(2):
                            nc.tensor.matmul(
                                psb[:, jt * 512:(jt + 1) * 512],
                                lhsT=lhsT,
                                rhs=rhs[:, kd, 1024 + jt * 512:1024 + (jt + 1) * 512],
                                start=(kd == 0), stop=(kd == KD - 1))
                    esa = esp.tile([P, 2 * 512], bf16, tag="esa")
                    nc.scalar.activation(
                        out=esa, in_=psa, func=AF.Exp,
                        scale=s1tau[:, ib:ib + 1],
                        accum_out=acc1[:, ib:ib + 1])
                    esb = esp.tile([P, 2 * 512], bf16, tag="esb", bufs=4)
                    nc.scalar.activation(
                        out=esb, in_=psb, func=AF.Exp,
                        scale=s1tau[:, ib:ib + 1],
                        accum_out=acc1b[:, ib:ib + 1])
                    esb_pend.append((ib, esb))
                    # colsum matmuls lag 2 iterations so PE never waits on ACT
                    if len(esb_pend) > 2:
                        _colsum(esb_pend.pop(0))
                for item in esb_pend:
                    _colsum(item)
                esb_pend = []

                for ib in range(NIB):
                    c, lb = divmod(ib, NZC)
                    ps = psp.tile([P, 2 * 512], f32, tag="ps", name="psc")
                    for kd in range(KD):
                        lhsT = zt_tiles["zt2"][kd][c][:, lb * P:(lb + 1) * P]
                        for jt in range(2):
                            nc.tensor.matmul(
                                ps[:, jt * 512:(jt + 1) * 512],
                                lhsT=lhsT,
                                rhs=rhs[:, kd, 1024 + jt * 512:1024 + (jt + 1) * 512],
                                start=(kd == 0), stop=(kd == KD - 1))
                    es = esp.tile([P, 2 * 512], bf16, tag="esa")
                    nc.scalar.activation(
                        out=es, in_=ps, func=AF.Exp,
                        scale=s2tau[:, ib:ib + 1],
                        accum_out=acc2[:, ib:ib + 1])

            if loop is not None:
                with tc.For_i(0, loop):
                    _main_body()
            else:
                for _rep in range(repeat):
                    _main_body()

            # combine refl1+between1 partials; stage colsums to SBUF
            nc.vector.tensor_add(acc1, acc1, acc1b)
            nc.vector.tensor_copy(ecol_s[0:1, 0:512], cs0[0:1, :])
            nc.vector.tensor_copy(ecol_s[0:1, 512:1024], cs1[0:1, :])
            nc.sync.dma_start(out=ecol[:].rearrange("(o c) -> o c", o=1),
                              in_=ecol_s)

            nc.sync.dma_start(
                out=partials[0].rearrange("(b p) -> p b", p=P), in_=acc1)
            nc.sync.dma_start(
                out=partials[1].rearrange("(b p) -> p b", p=P), in_=acc2)

    nc.compile()
    return nc


def _get_nc(repeat=1, loop=None):
    key = ("nc", repeat, loop)
    if key not in _CACHE:
        _CACHE[key] = _build(repeat, loop=loop)
    return _CACHE[key]


def make_in_maps(z1, z2):
    z1 = np.asarray(z1, dtype=np.float32)
    z2 = np.asarray(z2, dtype=np.float32)
    bf16 = ml_dtypes.bfloat16

    z1r = np.ascontiguousarray(z1.astype(bf16))
    z2r = np.ascontiguousarray(z2.astype(bf16))
    z1t = np.ascontiguousarray(z1r.T)
    z2t = np.ascontiguousarray(z2r.T)

    in_maps = []
    for r in range(NCORES):
        in_maps.append({
            "z1t": z1t, "z2t": z2t, "z1r": z1r, "z2r": z2r,
            "z1c": np.ascontiguousarray(z1r[r * CH:(r + 1) * CH]),
            "z2c": np.ascontiguousarray(z2r[r * CH:(r + 1) * CH]),
        })
    return in_maps


def kernel(z1, z2):
    from concourse.bass_utils import run_bass_kernel_spmd

    in_maps = make_in_maps(z1, z2)

    nc = _get_nc()
    res = run_bass_kernel_spmd(nc, in_maps, core_ids=list(range(NCORES)))

    S1 = np.zeros(N, dtype=np.float64)
    S2 = np.zeros(N, dtype=np.float64)
    v5 = np.zeros(N, dtype=np.float64)
    for r in range(NCORES):
        out = res.results[r]
        S1 += out["partials"][0].astype(np.float64)
        S2 += out["partials"][1].astype(np.float64)
        S2[r * CH:(r + 1) * CH] += out["ecol"].astype(np.float64)
        v5[r * CH:(r + 1) * CH] = out["diag"].astype(np.float64)

    e5 = np.exp(np.float64(TAU_INV))
    loss = 0.5 * (np.log(S1 - e5) + np.log(S2 - e5)) - v5
    return np.float32(loss.sum())



# revision 8
# speedup vs baseline: 1.5925x; 1.5925x over previous
"""Contrastive loss (GRACE-style) on 8 Trainium2 NeuronCores — fp8 edition.

loss = sum_i 0.5*(l1_i + l2_i)
  l1 = -log(diag(exp(h1@h2.T/t)) / (rowsum(exp(h1@h1.T/t)) + rowsum(exp(h1@h2.T/t)) - diag(exp(h1@h1.T/t))))
  l2 = same with h1<->h2;  h = z / ||z||_row,  t = 0.2

Sharding: columns (j) of the similarity matrices are sharded across 8 cores
(each core owns a 1024-column chunk of both h1 and h2). Each core computes,
for ALL 8192 rows i, the partial sums over its j-chunk of
exp(s_i * (z_i . h_j)), where the row normalization 1/(16*tau*||z_i||) is a
per-partition ACT scale. All matmuls run fp8(e4m3) DoubleRow (2x PE rate):
stationary = raw z.T tiles, moving = (h*16).T chunk tiles, contraction pairs
along the kd dimension. Per row-block: exp+rowsum of refl and between tiles
(ACT exp -> DVE reduce), plus ones-matmul colsums of exp'd between tiles
(= between.T rowsums for own chunk rows, PSUM-accumulated over all 64 row
blocks). Host (numpy, O(N*D)) prepares fp8 inputs/scales and applies the
exact diagonal corrections + logs in float64.
"""

import numpy as np
import ml_dtypes

N = 8192
D = 512
NCORES = 8
CH = N // NCORES  # 1024 columns per core
P = 128
KD = D // P  # 4 k-subtiles; DoubleRow consumes them in pairs
NIB = N // P  # 64 row blocks
TAU = 0.2
RS = 16.0  # rhs pre-scale to keep fp8 h values in the normal range

_CACHE = {}


def _build(repeat=1, loop=None):
    import concourse.tile as tile
    from concourse import bacc, mybir

    f32 = mybir.dt.float32
    bf16 = mybir.dt.bfloat16
    fp8 = mybir.dt.float8e4
    AF = mybir.ActivationFunctionType
    ALU = mybir.AluOpType
    DR = mybir.MatmulPerfMode.DoubleRow

    nc = bacc.Bacc("TRN2", target_bir_lowering=False, debug=False,
                   num_devices=NCORES)

    zt1 = nc.dram_tensor("zt1", [D, N], fp8, kind="ExternalInput")
    zt2 = nc.dram_tensor("zt2", [D, N], fp8, kind="ExternalInput")
    rh1 = nc.dram_tensor("rh1", [D, CH], fp8, kind="ExternalInput")
    rh2 = nc.dram_tensor("rh2", [D, CH], fp8, kind="ExternalInput")
    s1 = nc.dram_tensor("s1", [N], f32, kind="ExternalInput")
    s2 = nc.dram_tensor("s2", [N], f32, kind="ExternalInput")
    partials = nc.dram_tensor("partials", [2, N], f32, kind="ExternalOutput")
    ecol = nc.dram_tensor("ecol", [CH], f32, kind="ExternalOutput")

    zt1v = zt1.rearrange("(k p) n -> p k n", p=P)
    zt2v = zt2.rearrange("(k p) n -> p k n", p=P)
    rh1v = rh1.rearrange("(k p) n -> p k n", p=P)
    rh2v = rh2.rearrange("(k p) n -> p k n", p=P)

    with tile.TileContext(nc) as tc:
        with (
            tc.tile_pool(name="singles", bufs=1) as singles,
            tc.tile_pool(name="es", bufs=2) as esp,
            tc.tile_pool(name="ps", bufs=3, space="PSUM") as psp,
            tc.tile_pool(name="pscol", bufs=1, space="PSUM") as pscolp,
        ):
            # ---- persistent buffers ----
            zt1s = singles.tile([P, KD, N], fp8, tag="zt1s")
            zt2s = singles.tile([P, KD, N], fp8, tag="zt2s")
            rh1s = singles.tile([P, KD, CH], fp8, tag="rh1s")
            rh2s = singles.tile([P, KD, CH], fp8, tag="rh2s")
            s1s = singles.tile([P, NIB], f32, tag="s1s")
            s2s = singles.tile([P, NIB], f32, tag="s2s")
            acc1 = singles.tile([P, NIB], f32, tag="acc1")
            acc1b = singles.tile([P, NIB], f32, tag="acc1b")
            acc2 = singles.tile([P, NIB], f32, tag="acc2")
            ones = singles.tile([P, 1], bf16, tag="ones")
            nc.vector.memset(ones, 1.0)
            ecol_s = singles.tile([1, CH], f32, tag="ecol_s")
            cs = pscolp.tile([1, CH], f32, tag="cs")

            # ---- input DMAs (rhs + scales first: needed by every ib) ----
            nc.sync.dma_start(out=rh1s, in_=rh1v)
            nc.sync.dma_start(out=rh2s, in_=rh2v)
            nc.sync.dma_start(out=s1s, in_=s1.rearrange("(b p) -> p b", p=P))
            nc.sync.dma_start(out=s2s, in_=s2.rearrange("(b p) -> p b", p=P))
            nc.sync.dma_start(out=zt1s, in_=zt1v)
            nc.sync.dma_start(out=zt2s, in_=zt2v)

            # ---- main ----
            def _main_body():
                esb_pend = []

                def _colsum(item):
                    ibx, esb = item
                    for jt in range(2):
                        nc.tensor.matmul(
                            cs[0:1, jt * 512:(jt + 1) * 512],
                            lhsT=ones,
                            rhs=esb[:, jt * 512:(jt + 1) * 512],
                            start=(ibx == 0),
                            stop=(ibx == NIB - 1),
                            skip_group_check=True,
                        )

                # pass 1: z1 row blocks x [h1c | h2c] -> refl1 + between1
                for ib in range(NIB):
                    psa = psp.tile([P, 1024], f32, tag="ps", name="psa")
                    psb = psp.tile([P, 1024], f32, tag="ps", name="psb")
                    for kp in range(2):
                        lhsT = zt1s[:, 2 * kp:2 * kp + 2, ib * P:(ib + 1) * P]
                        for jt in range(2):
                            nc.tensor.matmul(
                                psa[:, jt * 512:(jt + 1) * 512],
                                lhsT=lhsT,
                                rhs=rh1s[:, 2 * kp:2 * kp + 2,
                                         jt * 512:(jt + 1) * 512],
                                start=(kp == 0), stop=(kp == 1),
                                perf_mode=DR)
                        for jt in range(2):
                            nc.tensor.matmul(
                                psb[:, jt * 512:(jt + 1) * 512],
                                lhsT=lhsT,
                                rhs=rh2s[:, 2 * kp:2 * kp + 2,
                                         jt * 512:(jt + 1) * 512],
                                start=(kp == 0), stop=(kp == 1),
                                perf_mode=DR)
                    esa = esp.tile([P, 1024], bf16, tag="esa")
                    nc.scalar.activation(out=esa, in_=psa, func=AF.Exp,
                                         scale=s1s[:, ib:ib + 1])
                    esb = esp.tile([P, 1024], bf16, tag="esb", bufs=4)
                    nc.scalar.activation(out=esb, in_=psb, func=AF.Exp,
                                         scale=s1s[:, ib:ib + 1])
                    nc.vector.tensor_reduce(acc1[:, ib:ib + 1], esa,
                                            axis=mybir.AxisListType.X,
                                            op=ALU.add)
                    nc.vector.tensor_reduce(acc1b[:, ib:ib + 1], esb,
                                            axis=mybir.AxisListType.X,
                                            op=ALU.add)
                    esb_pend.append((ib, esb))
                    # colsum matmuls lag 2 iterations so PE never waits on ACT
                    if len(esb_pend) > 2:
                        _colsum(esb_pend.pop(0))
                for item in esb_pend:
                    _colsum(item)

                # pass 2: z2 row blocks x h2c -> refl2
                for ib in range(NIB):
                    psc = psp.tile([P, 1024], f32, tag="ps", name="psc")
                    for kp in range(2):
                        lhsT = zt2s[:, 2 * kp:2 * kp + 2, ib * P:(ib + 1) * P]
                        for jt in range(2):
                            nc.tensor.matmul(
                                psc[:, jt * 512:(jt + 1) * 512],
                                lhsT=lhsT,
                                rhs=rh2s[:, 2 * kp:2 * kp + 2,
                                         jt * 512:(jt + 1) * 512],
                                start=(kp == 0), stop=(kp == 1),
                                perf_mode=DR)
                    esc = esp.tile([P, 1024], bf16, tag="esa")
                    nc.scalar.activation(out=esc, in_=psc, func=AF.Exp,
                                         scale=s2s[:, ib:ib + 1])
                    nc.vector.tensor_reduce(acc2[:, ib:ib + 1], esc,
                                            axis=mybir.AxisListType.X,
                                            op=ALU.add)

            if loop is not None:
                with tc.For_i(0, loop):
                    _main_body()
            else:
                for _rep in range(repeat):
                    _main_body()

            # combine refl1+between1 partials; stage colsums to SBUF
            nc.vector.tensor_add(acc1, acc1, acc1b)
            nc.vector.tensor_copy(ecol_s, cs)
            nc.sync.dma_start(out=ecol[:].rearrange("(o c) -> o c", o=1),
                              in_=ecol_s)
            nc.sync.dma_start(
                out=partials[0].rearrange("(b p) -> p b", p=P), in_=acc1)
            nc.sync.dma_start(
                out=partials[1].rearrange("(b p) -> p b", p=P), in_=acc2)

    nc.compile()
    return nc


def _get_nc(repeat=1, loop=None):
    key = ("nc", repeat, loop)
    if key not in _CACHE:
        _CACHE[key] = _build(repeat, loop=loop)
    return _CACHE[key]


def _host_prep(z1, z2):
    fp8 = ml_dtypes.float8_e4m3
    z1 = np.asarray(z1, dtype=np.float32)
    z2 = np.asarray(z2, dtype=np.float32)
    n1 = np.maximum(np.linalg.norm(z1, axis=1), 1e-12)
    n2 = np.maximum(np.linalg.norm(z2, axis=1), 1e-12)
    h1 = z1 / n1[:, None]
    h2 = z2 / n2[:, None]
    z1_8 = z1.astype(fp8)
    z2_8 = z2.astype(fp8)
    r1_8 = (h1 * RS).astype(fp8)
    r2_8 = (h2 * RS).astype(fp8)
    s1 = (1.0 / (RS * TAU * n1)).astype(np.float32)
    s2 = (1.0 / (RS * TAU * n2)).astype(np.float32)
    return z1_8, z2_8, r1_8, r2_8, s1, s2, h1, h2, n1, n2


def make_in_maps(z1, z2):
    z1_8, z2_8, r1_8, r2_8, s1, s2, _, _, _, _ = _host_prep(z1, z2)
    zt1 = np.ascontiguousarray(z1_8.T)
    zt2 = np.ascontiguousarray(z2_8.T)
    rt1 = r1_8.T
    rt2 = r2_8.T
    in_maps = []
    for r in range(NCORES):
        in_maps.append({
            "zt1": zt1, "zt2": zt2,
            "rh1": np.ascontiguousarray(rt1[:, r * CH:(r + 1) * CH]),
            "rh2": np.ascontiguousarray(rt2[:, r * CH:(r + 1) * CH]),
            "s1": s1, "s2": s2,
        })
    return in_maps


def kernel(z1, z2):
    from concourse.bass_utils import run_bass_kernel_spmd

    z1_8, z2_8, r1_8, r2_8, s1, s2, h1, h2, n1, n2 = _host_prep(z1, z2)
    in_maps = make_in_maps(z1, z2)

    nc = _get_nc()
    res = run_bass_kernel_spmd(nc, in_maps, core_ids=list(range(NCORES)))

    S1 = np.zeros(N, dtype=np.float64)
    S2 = np.zeros(N, dtype=np.float64)
    for r in range(NCORES):
        out = res.results[r]
        S1 += out["partials"][0].astype(np.float64)
        S2 += out["partials"][1].astype(np.float64)
        S2[r * CH:(r + 1) * CH] += out["ecol"].astype(np.float64)

    # exact diagonal corrections, computed from the same fp8 data the
    # device used: refl_ii = exp(s_i * (z8_i . r8_i))
    q1 = (z1_8.astype(np.float64) * r1_8.astype(np.float64)).sum(1) \
        * s1.astype(np.float64)
    q2 = (z2_8.astype(np.float64) * r2_8.astype(np.float64)).sum(1) \
        * s2.astype(np.float64)
    v5 = (h1.astype(np.float64) * h2.astype(np.float64)).sum(1) / TAU

    loss = 0.5 * (np.log(S1 - np.exp(q1)) + np.log(S2 - np.exp(q2))) - v5
    return np.float32(loss.sum())


# revision 13
# speedup vs baseline: 1.7991x; 1.1298x over previous
"""Contrastive loss (GRACE-style) on 8 Trainium2 NeuronCores — fp8 edition.

loss = sum_i 0.5*(l1_i + l2_i)
  l1 = -log(diag(exp(h1@h2.T/t)) / (rowsum(exp(h1@h1.T/t)) + rowsum(exp(h1@h2.T/t)) - diag(exp(h1@h1.T/t))))
  l2 = same with h1<->h2;  h = z / ||z||_row,  t = 0.2

Sharding: columns (j) of the similarity matrices are sharded across 8 cores
(each core owns a 1024-column chunk of both h1 and h2). Each core computes,
for ALL 8192 rows i, the partial sums over its j-chunk of
exp(s_i * (z_i . h_j)), where the row normalization 1/(16*tau*||z_i||) is a
per-partition ACT scale. All matmuls run fp8(e4m3) DoubleRow (2x PE rate):
stationary = raw z.T tiles, moving = (h*16).T chunk tiles, contraction pairs
along the kd dimension. Per row-block: exp+rowsum of refl and between tiles
(ACT exp -> DVE reduce), plus ones-matmul colsums of exp'd between tiles
(= between.T rowsums for own chunk rows, PSUM-accumulated over all 64 row
blocks). Host (numpy, O(N*D)) prepares fp8 inputs/scales and applies the
exact diagonal corrections + logs in float64.
"""

import numpy as np
import ml_dtypes

N = 8192
D = 512
NCORES = 8
CH = N // NCORES  # 1024 columns per core
P = 128
KD = D // P  # 4 k-subtiles; DoubleRow consumes them in pairs
NIB = N // P  # 64 row blocks
TAU = 0.2
RS = 16.0  # rhs pre-scale to keep fp8 h values in the normal range

_CACHE = {}


def _build(repeat=1, loop=None):
    import concourse.tile as tile
    from concourse import bacc, mybir

    f32 = mybir.dt.float32
    bf16 = mybir.dt.bfloat16
    fp8 = mybir.dt.float8e4
    AF = mybir.ActivationFunctionType
    ALU = mybir.AluOpType
    DR = mybir.MatmulPerfMode.DoubleRow

    nc = bacc.Bacc("TRN2", target_bir_lowering=False, debug=False,
                   num_devices=NCORES)

    zt1 = nc.dram_tensor("zt1", [D, N], fp8, kind="ExternalInput")
    zt2 = nc.dram_tensor("zt2", [D, N], fp8, kind="ExternalInput")
    rh1 = nc.dram_tensor("rh1", [D, CH], fp8, kind="ExternalInput")
    rh2 = nc.dram_tensor("rh2", [D, CH], fp8, kind="ExternalInput")
    s1 = nc.dram_tensor("s1", [N], f32, kind="ExternalInput")
    s2 = nc.dram_tensor("s2", [N], f32, kind="ExternalInput")
    partials = nc.dram_tensor("partials", [2, N], f32, kind="ExternalOutput")
    ecol = nc.dram_tensor("ecol", [CH], f32, kind="ExternalOutput")

    zt1v = zt1.rearrange("(k p) n -> p k n", p=P)
    zt2v = zt2.rearrange("(k p) n -> p k n", p=P)
    rh1v = rh1.rearrange("(k p) n -> p k n", p=P)
    rh2v = rh2.rearrange("(k p) n -> p k n", p=P)

    with tile.TileContext(nc) as tc:
        with (
            tc.tile_pool(name="singles", bufs=1) as singles,
            tc.tile_pool(name="es", bufs=2) as esp,
            tc.tile_pool(name="ps", bufs=3, space="PSUM") as psp,
            tc.tile_pool(name="pscol", bufs=1, space="PSUM") as pscolp,
        ):
            # ---- persistent buffers ----
            zt1s = singles.tile([P, KD, N], fp8, tag="zt1s")
            zt2s = singles.tile([P, KD, N], fp8, tag="zt2s")
            rh1s = singles.tile([P, KD, CH], fp8, tag="rh1s")
            rh2s = singles.tile([P, KD, CH], fp8, tag="rh2s")
            s1s = singles.tile([P, NIB], f32, tag="s1s")
            s2s = singles.tile([P, NIB], f32, tag="s2s")
            acc1 = singles.tile([P, NIB], f32, tag="acc1")
            acc2 = singles.tile([P, NIB], f32, tag="acc2")
            ones = singles.tile([P, 1], bf16, tag="ones")
            nc.vector.memset(ones, 1.0)
            ecol_s = singles.tile([1, CH], f32, tag="ecol_s")
            cs = pscolp.tile([1, CH], f32, tag="cs")

            # ---- input DMAs (rhs + scales first: needed by every ib) ----
            nc.sync.dma_start(out=rh1s, in_=rh1v)
            nc.sync.dma_start(out=rh2s, in_=rh2v)
            nc.sync.dma_start(out=s1s, in_=s1.rearrange("(b p) -> p b", p=P))
            nc.sync.dma_start(out=s2s, in_=s2.rearrange("(b p) -> p b", p=P))
            nc.sync.dma_start(out=zt1s, in_=zt1v)
            nc.sync.dma_start(out=zt2s, in_=zt2v)

            # ---- main ----
            def _main_body():
                esb_pend = []

                def _colsum(item):
                    ibx, esb = item
                    for jt in range(2):
                        nc.tensor.matmul(
                            cs[0:1, jt * 512:(jt + 1) * 512],
                            lhsT=ones,
                            rhs=esb[:, jt * 512:(jt + 1) * 512],
                            start=(ibx == 0),
                            stop=(ibx == NIB - 1),
                            skip_group_check=True,
                        )

                # pass 1: z1 row blocks x [h1c | h2c] -> refl1 + between1
                for ib in range(NIB):
                    psa = psp.tile([P, 1024], f32, tag="ps", name="psa")
                    psb = psp.tile([P, 1024], f32, tag="ps", name="psb")
                    for kp in range(2):
                        lhsT = zt1s[:, 2 * kp:2 * kp + 2, ib * P:(ib + 1) * P]
                        for jt in range(2):
                            nc.tensor.matmul(
                                psa[:, jt * 512:(jt + 1) * 512],
                                lhsT=lhsT,
                                rhs=rh1s[:, 2 * kp:2 * kp + 2,
                                         jt * 512:(jt + 1) * 512],
                                start=(kp == 0), stop=(kp == 1),
                                perf_mode=DR)
                        for jt in range(2):
                            nc.tensor.matmul(
                                psb[:, jt * 512:(jt + 1) * 512],
                                lhsT=lhsT,
                                rhs=rh2s[:, 2 * kp:2 * kp + 2,
                                         jt * 512:(jt + 1) * 512],
                                start=(kp == 0), stop=(kp == 1),
                                perf_mode=DR)
                    esa = esp.tile([P, 1024], bf16, tag="esa")
                    nc.scalar.activation(out=esa, in_=psa, func=AF.Exp,
                                         scale=s1s[:, ib:ib + 1])
                    esb = esp.tile([P, 1024], bf16, tag="esb", bufs=4)
                    nc.scalar.activation(out=esb, in_=psb, func=AF.Exp,
                                         scale=s1s[:, ib:ib + 1])
                    # refl1+between1 rowsums: 2x-rate add, then one 1x reduce
                    esum = esp.tile([P, 1024], bf16, tag="esum")
                    nc.vector.tensor_add(esum, esa, esb)
                    nc.vector.tensor_reduce(acc1[:, ib:ib + 1], esum,
                                            axis=mybir.AxisListType.X,
                                            op=ALU.add)
                    esb_pend.append((ib, esb))
                    # colsum matmuls lag 2 iterations so PE never waits on ACT
                    if len(esb_pend) > 2:
                        _colsum(esb_pend.pop(0))
                for item in esb_pend:
                    _colsum(item)

                # pass 2: z2 row blocks x h2c -> refl2
                for ib in range(NIB):
                    psc = psp.tile([P, 1024], f32, tag="ps", name="psc")
                    for kp in range(2):
                        lhsT = zt2s[:, 2 * kp:2 * kp + 2, ib * P:(ib + 1) * P]
                        for jt in range(2):
                            nc.tensor.matmul(
                                psc[:, jt * 512:(jt + 1) * 512],
                                lhsT=lhsT,
                                rhs=rh2s[:, 2 * kp:2 * kp + 2,
                                         jt * 512:(jt + 1) * 512],
                                start=(kp == 0), stop=(kp == 1),
                                perf_mode=DR)
                    esc = esp.tile([P, 1024], bf16, tag="esa")
                    nc.scalar.activation(out=esc, in_=psc, func=AF.Exp,
                                         scale=s2s[:, ib:ib + 1])
                    nc.vector.tensor_reduce(acc2[:, ib:ib + 1], esc,
                                            axis=mybir.AxisListType.X,
                                            op=ALU.add)

            if loop is not None:
                with tc.For_i(0, loop):
                    _main_body()
            else:
                for _rep in range(repeat):
                    _main_body()

            # stage colsums to SBUF
            nc.vector.tensor_copy(ecol_s, cs)
            nc.sync.dma_start(out=ecol[:].rearrange("(o c) -> o c", o=1),
                              in_=ecol_s)
            nc.sync.dma_start(
                out=partials[0].rearrange("(b p) -> p b", p=P), in_=acc1)
            nc.sync.dma_start(
                out=partials[1].rearrange("(b p) -> p b", p=P), in_=acc2)

    nc.compile()
    return nc


def _get_nc(repeat=1, loop=None):
    key = ("nc", repeat, loop)
    if key not in _CACHE:
        _CACHE[key] = _build(repeat, loop=loop)
    return _CACHE[key]


def _host_prep(z1, z2):
    fp8 = ml_dtypes.float8_e4m3
    z1 = np.asarray(z1, dtype=np.float32)
    z2 = np.asarray(z2, dtype=np.float32)
    n1 = np.maximum(np.linalg.norm(z1, axis=1), 1e-12)
    n2 = np.maximum(np.linalg.norm(z2, axis=1), 1e-12)
    h1 = z1 / n1[:, None]
    h2 = z2 / n2[:, None]
    z1_8 = z1.astype(fp8)
    z2_8 = z2.astype(fp8)
    r1_8 = (h1 * RS).astype(fp8)
    r2_8 = (h2 * RS).astype(fp8)
    s1 = (1.0 / (RS * TAU * n1)).astype(np.float32)
    s2 = (1.0 / (RS * TAU * n2)).astype(np.float32)
    return z1_8, z2_8, r1_8, r2_8, s1, s2, h1, h2, n1, n2


def make_in_maps(z1, z2):
    z1_8, z2_8, r1_8, r2_8, s1, s2, _, _, _, _ = _host_prep(z1, z2)
    zt1 = np.ascontiguousarray(z1_8.T)
    zt2 = np.ascontiguousarray(z2_8.T)
    rt1 = r1_8.T
    rt2 = r2_8.T
    in_maps = []
    for r in range(NCORES):
        in_maps.append({
            "zt1": zt1, "zt2": zt2,
            "rh1": np.ascontiguousarray(rt1[:, r * CH:(r + 1) * CH]),
            "rh2": np.ascontiguousarray(rt2[:, r * CH:(r + 1) * CH]),
            "s1": s1, "s2": s2,
        })
    return in_maps


def kernel(z1, z2):
    from concourse.bass_utils import run_bass_kernel_spmd

    z1_8, z2_8, r1_8, r2_8, s1, s2, h1, h2, n1, n2 = _host_prep(z1, z2)
    in_maps = make_in_maps(z1, z2)

    nc = _get_nc()
    res = run_bass_kernel_spmd(nc, in_maps, core_ids=list(range(NCORES)))

    S1 = np.zeros(N, dtype=np.float64)
    S2 = np.zeros(N, dtype=np.float64)
    for r in range(NCORES):
        out = res.results[r]
        S1 += out["partials"][0].astype(np.float64)
        S2 += out["partials"][1].astype(np.float64)
        S2[r * CH:(r + 1) * CH] += out["ecol"].astype(np.float64)

    # exact diagonal corrections, computed from the same fp8 data the
    # device used: refl_ii = exp(s_i * (z8_i . r8_i))
    q1 = (z1_8.astype(np.float64) * r1_8.astype(np.float64)).sum(1) \
        * s1.astype(np.float64)
    q2 = (z2_8.astype(np.float64) * r2_8.astype(np.float64)).sum(1) \
        * s2.astype(np.float64)
    v5 = (h1.astype(np.float64) * h2.astype(np.float64)).sum(1) / TAU

    loss = 0.5 * (np.log(S1 - np.exp(q1)) + np.log(S2 - np.exp(q2))) - v5
    return np.float32(loss.sum())
